# revision 17
# baseline (speedup 1.0000x reference)
"""Cross-attention Trainium2 kernel (8 NeuronCores, SPMD, no collectives).

Reference computation (f32):
    Q = tokens @ Wq; K = context @ Wk; V = context @ Wv    (per batch)
    attn = softmax(Q K^T / sqrt(64)); ctx = attn V; out = ctx @ Wo + bo

Sharding: the flattened (B*T = 16384) token rows are split into 8 slices of
2048; each slice lies inside a single batch, so each core computes its own
batch's K/V locally (context is small) and needs no cross-core traffic.

Layout trick: all activations are kept contraction-major ("transposed") so
every matmul's operands are naturally [K, M]/[K, N]:
  - host supplies tokens^T and context^T slices (bf16)
  - Q^T/K^T are produced directly in [EMB, seq] layout
  - scores are produced transposed ([S, T]) so the PV matmul needs no
    transpose; the softmax row-sum comes from a ones-column appended to V.

Schedule: the (tn, head-pair) loop is flattened into one group list and
software-pipelined one group deep: group i issues its score matmuls + exps
while the PV accumulation of group i-1 interleaves at s8 granularity, and
out-proj (tn-1) / Q-proj (tn+1) chunks fill the remaining PE slots.  This
keeps the PE free of >3.4us idle gaps (HAM stays at K=8/8, 2.4 GHz) and
hides the Scalar-engine exp latency behind PE work.
"""

import numpy as np
import ml_dtypes

import concourse.bass as bass
import concourse.mybir as mybir
import concourse.tile as tile
from concourse import bacc
from concourse.bass_utils import run_bass_kernel_spmd

# problem shapes (hardcoded per the contract)
B, T, S = 4, 4096, 1024
HID, EMB, CTX, H = 1024, 1024, 768, 16
D = EMB // H  # 64
N_CORES = 8
TC = (B * T) // N_CORES  # 2048 token rows per core

F32 = mybir.dt.float32
BF16 = mybir.dt.bfloat16
AF = mybir.ActivationFunctionType

_BUILT = {}


def _build_nc(tc_=TC, s_=S, ctx_=CTX, hid_=HID, h_=H, num_cores=N_CORES):
    nc = bacc.Bacc("TRN2", target_bir_lowering=False, debug=False,
                   num_devices=num_cores)
    emb_ = hid_

    tokT = nc.dram_tensor("tokT", [hid_, tc_], BF16, kind="ExternalInput")
    ctxT = nc.dram_tensor("ctxT", [ctx_, s_], BF16, kind="ExternalInput")
    wq = nc.dram_tensor("wq", [hid_, emb_], BF16, kind="ExternalInput")
    wk = nc.dram_tensor("wk", [ctx_, emb_], BF16, kind="ExternalInput")
    wv = nc.dram_tensor("wv", [ctx_, hid_], BF16, kind="ExternalInput")
    wo = nc.dram_tensor("wo", [emb_, hid_], BF16, kind="ExternalInput")
    bo = nc.dram_tensor("bo", [1, hid_], F32, kind="ExternalInput")
    out = nc.dram_tensor("out", [tc_, hid_], F32, kind="ExternalOutput")

    K8 = hid_ // 128   # contraction chunks for Q proj
    C6 = ctx_ // 128   # contraction chunks for K/V proj
    E8 = emb_ // 128   # embedding chunks
    S8 = s_ // 128     # source-sequence chunks
    TN = tc_ // 512    # T chunks per core
    NH = hid_ // 512   # output free chunks
    HID, EMB, CTX, S, TC, H = hid_, emb_, ctx_, s_, tc_, h_  # noqa: shadow
    NP = H // 2        # head pairs per T chunk

    with tile.TileContext(nc) as tc:
        with (
            tc.tile_pool(name="const", bufs=1) as const,
            tc.tile_pool(name="wkvo", bufs=12) as wkvo,
            tc.tile_pool(name="qpool", bufs=2) as qpool,
            tc.tile_pool(name="cpool", bufs=2) as cpool,
            tc.tile_pool(name="attn", bufs=10) as attnp,
            tc.tile_pool(name="small", bufs=3) as small,
            tc.tile_pool(name="ostage", bufs=3) as ostage,
            tc.tile_pool(name="spsum", bufs=2, space="PSUM") as spsum,
            tc.tile_pool(name="cpsum", bufs=2, space="PSUM") as cpsum,
            tc.tile_pool(name="ppsum", bufs=2, space="PSUM") as ppsum,
        ):
            # ---- resident inputs --------------------------------------
            # DMA issue order = arrival order; the K-proj dependencies
            # (ctxT, wk) go first, split across idle engine queues so the
            # transfers ride parallel DMA rings.  Everything needed later
            # (wq, tokens, wo) streams behind them.
            # each transfer is split into 4 sub-DMAs: one DMA serializes on
            # a single DMA engine (~21 GB/s); the DGE round-robins separate
            # DMAs across the 16 engines, so 4 splits arrive ~4x sooner.
            def dma4(q, t, src_ap, cols):
                for j in range(4):
                    fs = slice(j * (cols // 4), (j + 1) * (cols // 4))
                    q(out=t[:, fs], in_=src_ap[:, fs])

            wk_t, wv_t, ctxT_t = [], [], []
            for c in range(C6):
                t = const.tile([128, S], BF16, name=f"ctxT{c}")
                dma4(nc.sync.dma_start, t, ctxT[c * 128:(c + 1) * 128, :], S)
                ctxT_t.append(t)
            for c in range(C6):
                t = wkvo.tile([128, EMB], BF16, tag="wkvo")
                dma4(nc.scalar.dma_start, t, wk[c * 128:(c + 1) * 128, :], EMB)
                wk_t.append(t)
            for c in range(C6):
                t = wkvo.tile([128, HID], BF16, tag="wkvo")
                dma4(nc.gpsimd.dma_start, t, wv[c * 128:(c + 1) * 128, :], HID)
                wv_t.append(t)
            wq_t = []
            for k in range(K8):
                t = const.tile([128, EMB], BF16, name=f"wq{k}")
                dma4(nc.scalar.dma_start, t, wq[k * 128:(k + 1) * 128, :], EMB)
                wq_t.append(t)
            tokT_t = []
            for k in range(K8):
                t = const.tile([128, TC], BF16, name=f"tokT{k}")
                dma4(nc.sync.dma_start, t, tokT[k * 128:(k + 1) * 128, :], TC)
                tokT_t.append(t)
            bo_bc = const.tile([128, HID], F32)
            nc.gpsimd.dma_start(out=bo_bc, in_=bo[0:1, :].to_broadcast((128, HID)))

            # ---- PE warm-up -------------------------------------------
            # ~40 dummy matmuls on a memset tile, issued before any real
            # work and with no DMA dependency: sustained PE activity makes
            # HAM un-throttle (K=4/8 -> K=8/8, 1.2 -> 2.4 GHz) during the
            # input-DMA wait instead of 20us into the K/V projections.
            # The accumulator bank is never read.
            warm = const.tile([128, 512], BF16)
            nc.vector.memset(warm, 0.001)
            wps = ppsum.tile([128, 512], F32, tag="proj")
            for i in range(20):
                nc.tensor.matmul(wps, lhsT=warm[:, 0:128], rhs=warm,
                                 start=(i == 0), stop=(i == 19))

            # ---- K^T = (context @ Wk)^T : [EMB, S] --------------------
            kT_sb = const.tile([128, E8, S], BF16)
            for e in range(E8):
                for n in range(S // 512):
                    ps = ppsum.tile([128, 512], F32, tag="proj")
                    for c in range(C6):
                        nc.tensor.matmul(
                            ps,
                            lhsT=wk_t[c][:, e * 128:(e + 1) * 128],
                            rhs=ctxT_t[c][:, n * 512:(n + 1) * 512],
                            start=(c == 0), stop=(c == C6 - 1),
                        )
                    nc.vector.tensor_copy(kT_sb[:, e, n * 512:(n + 1) * 512], ps)

            # ---- V (+ 64-wide ones block) -----------------------------
            # per (s-chunk, head): a [128, 128] lhsT whose one half is the
            # head's V columns and the other half is ones, parity-placed so
            # the PV matmul emits ctx rows at partitions par*64..par*64+63
            # and the sumexp (replicated 64x) in the other half.
            # layout: [128, s8, pair, par, 128]; head h = (pair=h//2, par=h%2)
            # par==0: cols 0:64 = V, 64:128 = ones
            # par==1: cols 0:64 = ones, 64:128 = V
            v_sb = const.tile([128, S8, NP, 2, 128], BF16)
            for s in range(S8):
                nc.vector.memset(v_sb[:, s, :, 0, D:2 * D], 1.0)
                nc.vector.memset(v_sb[:, s, :, 1, 0:D], 1.0)
            for s in range(S8):
                for n in range(NH):
                    ps = ppsum.tile([128, 512], F32, tag="proj")
                    for c in range(C6):
                        nc.tensor.matmul(
                            ps,
                            lhsT=ctxT_t[c][:, s * 128:(s + 1) * 128],
                            rhs=wv_t[c][:, n * 512:(n + 1) * 512],
                            start=(c == 0), stop=(c == C6 - 1),
                        )
                    psv = ps.rearrange("p (pr two d) -> p pr two d", two=2, d=D)
                    pr = slice(n * 4, (n + 1) * 4)
                    nc.vector.tensor_copy(v_sb[:, s, pr, 0, 0:D], psv[:, :, 0, :])
                    nc.vector.tensor_copy(v_sb[:, s, pr, 1, D:2 * D], psv[:, :, 1, :])

            # ---- Wo loads (slots freed by wk/wv as KV phase drains) ---
            wo_t = []
            for e in range(E8):
                t = wkvo.tile([128, HID], BF16, tag="wkvo")
                nc.sync.dma_start(out=t, in_=wo[e * 128:(e + 1) * 128, :])
                wo_t.append(t)

            # ---- helpers ----------------------------------------------
            def qproj_chunk(qt, tn, e):
                """Q^T e-chunk for T chunk tn: qt[:, e, :] = (Wq^T tok)^T."""
                tsl = slice(tn * 512, (tn + 1) * 512)
                ps = ppsum.tile([128, 512], F32, tag="proj")
                for k in range(K8):
                    nc.tensor.matmul(
                        ps,
                        lhsT=wq_t[k][:, e * 128:(e + 1) * 128],
                        rhs=tokT_t[k][:, tsl],
                        start=(k == 0), stop=(k == K8 - 1),
                    )
                nc.vector.tensor_copy(qt[:, e, :], ps)

            def outproj_chunk(ct, tn, m, n):
                """out[tn*512+m*128 : .., n*512 : ..] = ct^T.T @ Wo + bo."""
                ps = ppsum.tile([128, 512], F32, tag="proj")
                for e in range(E8):
                    nc.tensor.matmul(
                        ps,
                        lhsT=ct[:, e, m * 128:(m + 1) * 128],
                        rhs=wo_t[e][:, n * 512:(n + 1) * 512],
                        start=(e == 0), stop=(e == E8 - 1),
                    )
                ot = ostage.tile([128, 512], F32, tag="ot")
                nc.vector.tensor_add(ot, ps, bo_bc[:, n * 512:(n + 1) * 512])
                nc.sync.dma_start(
                    out=out[tn * 512 + m * 128: tn * 512 + (m + 1) * 128,
                            n * 512:(n + 1) * 512],
                    in_=ot,
                )

            def scores_exp(qt, p, s8):
                """Two-head score matmuls (disjoint PE row halves) + exp."""
                sp = spsum.tile([128, 2, 512], F32, tag="sp")
                for par in range(2):
                    prow = slice(par * 64, par * 64 + 64)
                    nc.tensor.matmul(
                        sp[:, par, :],
                        lhsT=kT_sb[prow, p, s8 * 128:(s8 + 1) * 128],
                        rhs=qt[prow, p, :],
                        start=True, stop=True,
                    )
                at = attnp.tile([128, 2, 512], BF16, tag="at")
                nc.scalar.activation(at, sp, AF.Exp, scale=0.125)
                return at

            def pv_step(cps, at_tiles, p, s8):
                """One s8 accumulation step of the PV matmul, both parities."""
                for par in range(2):
                    nc.tensor.matmul(
                        cps[par],
                        lhsT=v_sb[:, s8, p, par, :],
                        rhs=at_tiles[s8][:, par, :],
                        start=(s8 == 0), stop=(s8 == S8 - 1),
                        skip_group_check=True,
                    )

            def normalize(ct, cps, p):
                """ct[:, p, :] = ctx rows / sumexp (per parity)."""
                for par in range(2):
                    cp = cps[par]
                    crow = slice(par * 64, par * 64 + 64)
                    srow = slice(64 - par * 64, 128 - par * 64)
                    rbs = small.tile([128, 512], F32, tag="rbs")
                    rb = small.tile([128, 512], F32, tag="rb")
                    # full-tile: the custom DVE op mis-addresses partition-
                    # offset APs; the ctx-half results are simply unused
                    nc.vector.reciprocal_approx_fast(rbs, cp)
                    nc.sync.dma_start(out=rb[crow, :], in_=rbs[srow, :])
                    nc.vector.tensor_mul(ct[crow, p, :], cp[crow, :], rb[crow, :])

            # ---- prologue: qt(0) --------------------------------------
            qts = [qpool.tile([128, E8, 512], BF16, tag="qt", name=f"qt{i}")
                   for i in range(2)]
            cts = [cpool.tile([128, E8, 512], BF16, tag="ct", name=f"ct{i}")
                   for i in range(2)]
            for e in range(E8):
                qproj_chunk(qts[0], 0, e)

            # ---- software-pipelined main loop -------------------------
            # groups: (tn, p); group i issues its scores/exp while group
            # i-1's PV accumulates between them; out-proj of tn-1 and
            # Q-proj of tn+1 chunks fill remaining PE slots.
            groups = [(tn, p) for tn in range(TN) for p in range(NP)]
            out_q = []    # pending outproj chunks: (ct, tn, m, n)
            qp_q = []     # pending qproj chunks: (qt, tn, e)
            prev = None   # (p_prev, at_tiles, cps, ct_prev)

            for tn, p in groups:
                qt = qts[tn % 2]
                ct = cts[tn % 2]
                if p == 0:
                    if tn + 1 < TN:
                        qp_q.extend((qts[(tn + 1) % 2], tn + 1, e)
                                    for e in range(E8))
                    if tn >= 1:
                        out_q.extend((cts[(tn - 1) % 2], tn - 1, m, n)
                                     for m in range(4) for n in range(NH))

                at_tiles = []
                cps = None
                if prev is not None:
                    cps = [cpsum.tile([128, 512], F32, tag="cp",
                                      name=f"cp_{tn}_{p}_{i}") for i in range(2)]

                # s8=0 scores first, then the Q-proj filler (no dependency
                # on this tn's ct) absorbs the cpsum recycle latency before
                # the first PV step needs the banks
                at_tiles.append(scores_exp(qt, p, 0))
                if qp_q:
                    qproj_chunk(*qp_q.pop(0))
                for s8 in range(1, S8):
                    if prev is not None:
                        pv_step(cps, prev[1], prev[0], s8 - 1)
                    at_tiles.append(scores_exp(qt, p, s8))
                if prev is not None:
                    pv_step(cps, prev[1], prev[0], S8 - 1)
                    normalize(prev[2], cps, prev[0])
                # out-proj chunks of tn-1 are only safe after normalize of
                # the pair that completed this group (they contract over
                # every pair), so they fill the post-normalize slot
                if out_q:
                    outproj_chunk(*out_q.pop(0))
                prev = (p, at_tiles, ct)

            # ---- epilogue: last group's PV + final out-proj -----------
            cps = [cpsum.tile([128, 512], F32, tag="cp", name=f"cp_last_{i}")
                   for i in range(2)]
            for s8 in range(S8):
                pv_step(cps, prev[1], prev[0], s8)
            normalize(prev[2], cps, prev[0])
            for m in range(4):
                for n in range(NH):
                    outproj_chunk(cts[(TN - 1) % 2], TN - 1, m, n)

    nc.compile()
    return nc


def _get_nc():
    if "nc" not in _BUILT:
        _BUILT["nc"] = _build_nc()
    return _BUILT["nc"]


def _bf16(x):
    return np.asarray(x, dtype=np.float32).astype(ml_dtypes.bfloat16)


def kernel(tokens, context, Wq, Wk, Wv, Wo, bo):
    tokens = np.asarray(tokens, dtype=np.float32).reshape(B * T, HID)
    context = np.asarray(context, dtype=np.float32)
    bo2 = np.asarray(bo, dtype=np.float32).reshape(1, HID)
    wq_b, wk_b, wv_b, wo_b = _bf16(Wq), _bf16(Wk), _bf16(Wv), _bf16(Wo)

    in_maps = []
    for c in range(N_CORES):
        b = (c * TC) // T
        tok_slice = tokens[c * TC:(c + 1) * TC, :]
        in_maps.append({
            "tokT": np.ascontiguousarray(tok_slice.T).astype(ml_dtypes.bfloat16),
            "ctxT": np.ascontiguousarray(context[b].T).astype(ml_dtypes.bfloat16),
            "wq": wq_b, "wk": wk_b, "wv": wv_b, "wo": wo_b, "bo": bo2,
        })

    nc = _get_nc()
    res = run_bass_kernel_spmd(nc, in_maps, list(range(N_CORES)))
    out = np.concatenate([res.results[c]["out"] for c in range(N_CORES)], axis=0)
    return out.reshape(B, T, HID)


# revision 18
# speedup vs baseline: 1.0176x; 1.0176x over previous
"""Cross-attention Trainium2 kernel (8 NeuronCores, SPMD, no collectives).

Reference computation (f32):
    Q = tokens @ Wq; K = context @ Wk; V = context @ Wv    (per batch)
    attn = softmax(Q K^T / sqrt(64)); ctx = attn V; out = ctx @ Wo + bo

Sharding: the flattened (B*T = 16384) token rows are split into 8 slices of
2048; each slice lies inside a single batch, so each core computes its own
batch's K/V locally (context is small) and needs no cross-core traffic.

Layout trick: all activations are kept contraction-major ("transposed") so
every matmul's operands are naturally [K, M]/[K, N]:
  - host supplies tokens^T and context^T slices (bf16)
  - Q^T/K^T are produced directly in [EMB, seq] layout
  - scores are produced transposed ([S, T]) so the PV matmul needs no
    transpose; the softmax row-sum comes from a ones-column appended to V.

Schedule: the (tn, head-pair) loop is flattened into one group list and
software-pipelined one group deep: group i issues its score matmuls + exps
while the PV accumulation of group i-1 interleaves at s8 granularity, and
out-proj (tn-1) / Q-proj (tn+1) chunks fill the remaining PE slots.  This
keeps the PE free of >3.4us idle gaps (HAM stays at K=8/8, 2.4 GHz) and
hides the Scalar-engine exp latency behind PE work.
"""

import numpy as np
import ml_dtypes

import concourse.bass as bass
import concourse.mybir as mybir
import concourse.tile as tile
from concourse import bacc
from concourse.bass_utils import run_bass_kernel_spmd

# problem shapes (hardcoded per the contract)
B, T, S = 4, 4096, 1024
HID, EMB, CTX, H = 1024, 1024, 768, 16
D = EMB // H  # 64
N_CORES = 8
TC = (B * T) // N_CORES  # 2048 token rows per core

F32 = mybir.dt.float32
BF16 = mybir.dt.bfloat16
AF = mybir.ActivationFunctionType

_BUILT = {}


def _build_nc(tc_=TC, s_=S, ctx_=CTX, hid_=HID, h_=H, num_cores=N_CORES):
    nc = bacc.Bacc("TRN2", target_bir_lowering=False, debug=False,
                   num_devices=num_cores)
    emb_ = hid_

    tokT = nc.dram_tensor("tokT", [hid_, tc_], BF16, kind="ExternalInput")
    ctxT = nc.dram_tensor("ctxT", [ctx_, s_], BF16, kind="ExternalInput")
    wq = nc.dram_tensor("wq", [hid_, emb_], BF16, kind="ExternalInput")
    wk = nc.dram_tensor("wk", [ctx_, emb_], BF16, kind="ExternalInput")
    wv = nc.dram_tensor("wv", [ctx_, hid_], BF16, kind="ExternalInput")
    wo = nc.dram_tensor("wo", [emb_, hid_], BF16, kind="ExternalInput")
    bo = nc.dram_tensor("bo", [1, hid_], F32, kind="ExternalInput")
    out = nc.dram_tensor("out", [tc_, hid_], F32, kind="ExternalOutput")

    K8 = hid_ // 128   # contraction chunks for Q proj
    C6 = ctx_ // 128   # contraction chunks for K/V proj
    E8 = emb_ // 128   # embedding chunks
    S8 = s_ // 128     # source-sequence chunks
    TN = tc_ // 512    # T chunks per core
    NH = hid_ // 512   # output free chunks
    HID, EMB, CTX, S, TC, H = hid_, emb_, ctx_, s_, tc_, h_  # noqa: shadow
    NP = H // 2        # head pairs per T chunk

    with tile.TileContext(nc) as tc:
        with (
            tc.tile_pool(name="const", bufs=1) as const,
            tc.tile_pool(name="wkvo", bufs=12) as wkvo,
            tc.tile_pool(name="qpool", bufs=2) as qpool,
            tc.tile_pool(name="cpool", bufs=2) as cpool,
            tc.tile_pool(name="attn", bufs=10) as attnp,
            tc.tile_pool(name="small", bufs=3) as small,
            tc.tile_pool(name="ostage", bufs=3) as ostage,
            tc.tile_pool(name="spsum", bufs=2, space="PSUM") as spsum,
            tc.tile_pool(name="cpsum", bufs=2, space="PSUM") as cpsum,
            tc.tile_pool(name="ppsum", bufs=2, space="PSUM") as ppsum,
        ):
            # ---- resident inputs --------------------------------------
            # DMA issue order = arrival order; the K-proj dependencies
            # (ctxT, wk) go first, split across idle engine queues so the
            # transfers ride parallel DMA rings.  Everything needed later
            # (wq, tokens, wo) streams behind them.
            wk_t, wv_t, ctxT_t = [], [], []
            for c in range(C6):
                t = const.tile([128, S], BF16, name=f"ctxT{c}")
                nc.sync.dma_start(out=t, in_=ctxT[c * 128:(c + 1) * 128, :])
                ctxT_t.append(t)
            for c in range(C6):
                t = wkvo.tile([128, EMB], BF16, tag="wkvo")
                nc.scalar.dma_start(out=t, in_=wk[c * 128:(c + 1) * 128, :])
                wk_t.append(t)
            for c in range(C6):
                t = wkvo.tile([128, HID], BF16, tag="wkvo")
                nc.gpsimd.dma_start(out=t, in_=wv[c * 128:(c + 1) * 128, :])
                wv_t.append(t)
            wq_t = []
            for k in range(K8):
                t = const.tile([128, EMB], BF16, name=f"wq{k}")
                nc.scalar.dma_start(out=t, in_=wq[k * 128:(k + 1) * 128, :])
                wq_t.append(t)
            tokT_t = []
            for k in range(K8):
                t = const.tile([128, TC], BF16, name=f"tokT{k}")
                nc.sync.dma_start(out=t, in_=tokT[k * 128:(k + 1) * 128, :])
                tokT_t.append(t)
            bo_bc = const.tile([128, HID], F32)
            nc.gpsimd.dma_start(out=bo_bc, in_=bo[0:1, :].to_broadcast((128, HID)))

            # ---- K^T = (context @ Wk)^T : [EMB, S] --------------------
            kT_sb = const.tile([128, E8, S], BF16)
            for e in range(E8):
                for n in range(S // 512):
                    ps = ppsum.tile([128, 512], F32, tag="proj")
                    for c in range(C6):
                        nc.tensor.matmul(
                            ps,
                            lhsT=wk_t[c][:, e * 128:(e + 1) * 128],
                            rhs=ctxT_t[c][:, n * 512:(n + 1) * 512],
                            start=(c == 0), stop=(c == C6 - 1),
                        )
                    nc.vector.tensor_copy(kT_sb[:, e, n * 512:(n + 1) * 512], ps)

            # ---- V (+ 64-wide ones block) -----------------------------
            # per (s-chunk, head): a [128, 128] lhsT whose one half is the
            # head's V columns and the other half is ones, parity-placed so
            # the PV matmul emits ctx rows at partitions par*64..par*64+63
            # and the sumexp (replicated 64x) in the other half.
            # layout: [128, s8, pair, par, 128]; head h = (pair=h//2, par=h%2)
            # par==0: cols 0:64 = V, 64:128 = ones
            # par==1: cols 0:64 = ones, 64:128 = V
            v_sb = const.tile([128, S8, NP, 2, 128], BF16)
            for s in range(S8):
                nc.vector.memset(v_sb[:, s, :, 0, D:2 * D], 1.0)
                nc.vector.memset(v_sb[:, s, :, 1, 0:D], 1.0)
            for s in range(S8):
                for n in range(NH):
                    ps = ppsum.tile([128, 512], F32, tag="proj")
                    for c in range(C6):
                        nc.tensor.matmul(
                            ps,
                            lhsT=ctxT_t[c][:, s * 128:(s + 1) * 128],
                            rhs=wv_t[c][:, n * 512:(n + 1) * 512],
                            start=(c == 0), stop=(c == C6 - 1),
                        )
                    psv = ps.rearrange("p (pr two d) -> p pr two d", two=2, d=D)
                    pr = slice(n * 4, (n + 1) * 4)
                    nc.vector.tensor_copy(v_sb[:, s, pr, 0, 0:D], psv[:, :, 0, :])
                    nc.vector.tensor_copy(v_sb[:, s, pr, 1, D:2 * D], psv[:, :, 1, :])

            # ---- Wo loads (slots freed by wk/wv as KV phase drains) ---
            wo_t = []
            for e in range(E8):
                t = wkvo.tile([128, HID], BF16, tag="wkvo")
                nc.sync.dma_start(out=t, in_=wo[e * 128:(e + 1) * 128, :])
                wo_t.append(t)

            # ---- helpers ----------------------------------------------
            def qproj_chunk(qt, tn, e):
                """Q^T e-chunk for T chunk tn: qt[:, e, :] = (Wq^T tok)^T."""
                tsl = slice(tn * 512, (tn + 1) * 512)
                ps = ppsum.tile([128, 512], F32, tag="proj")
                for k in range(K8):
                    nc.tensor.matmul(
                        ps,
                        lhsT=wq_t[k][:, e * 128:(e + 1) * 128],
                        rhs=tokT_t[k][:, tsl],
                        start=(k == 0), stop=(k == K8 - 1),
                    )
                nc.vector.tensor_copy(qt[:, e, :], ps)

            def outproj_chunk(ct, tn, m, n):
                """out[tn*512+m*128 : .., n*512 : ..] = ct^T.T @ Wo + bo."""
                ps = ppsum.tile([128, 512], F32, tag="proj")
                for e in range(E8):
                    nc.tensor.matmul(
                        ps,
                        lhsT=ct[:, e, m * 128:(m + 1) * 128],
                        rhs=wo_t[e][:, n * 512:(n + 1) * 512],
                        start=(e == 0), stop=(e == E8 - 1),
                    )
                ot = ostage.tile([128, 512], F32, tag="ot")
                nc.vector.tensor_add(ot, ps, bo_bc[:, n * 512:(n + 1) * 512])
                nc.sync.dma_start(
                    out=out[tn * 512 + m * 128: tn * 512 + (m + 1) * 128,
                            n * 512:(n + 1) * 512],
                    in_=ot,
                )

            def scores_exp(qt, p, s8):
                """Two-head score matmuls (disjoint PE row halves) + exp."""
                sp = spsum.tile([128, 2, 512], F32, tag="sp")
                for par in range(2):
                    prow = slice(par * 64, par * 64 + 64)
                    nc.tensor.matmul(
                        sp[:, par, :],
                        lhsT=kT_sb[prow, p, s8 * 128:(s8 + 1) * 128],
                        rhs=qt[prow, p, :],
                        start=True, stop=True,
                    )
                at = attnp.tile([128, 2, 512], BF16, tag="at")
                nc.scalar.activation(at, sp, AF.Exp, scale=0.125)
                return at

            def pv_step(cps, at_tiles, p, s8):
                """One s8 accumulation step of the PV matmul, both parities."""
                for par in range(2):
                    nc.tensor.matmul(
                        cps[par],
                        lhsT=v_sb[:, s8, p, par, :],
                        rhs=at_tiles[s8][:, par, :],
                        start=(s8 == 0), stop=(s8 == S8 - 1),
                        skip_group_check=True,
                    )

            def normalize(ct, cps, p):
                """ct[:, p, :] = ctx rows / sumexp (per parity)."""
                for par in range(2):
                    cp = cps[par]
                    crow = slice(par * 64, par * 64 + 64)
                    srow = slice(64 - par * 64, 128 - par * 64)
                    rbs = small.tile([128, 512], F32, tag="rbs")
                    rb = small.tile([128, 512], F32, tag="rb")
                    # full-tile: the custom DVE op mis-addresses partition-
                    # offset APs; the ctx-half results are simply unused
                    nc.vector.reciprocal_approx_fast(rbs, cp)
                    nc.sync.dma_start(out=rb[crow, :], in_=rbs[srow, :])
                    nc.vector.tensor_mul(ct[crow, p, :], cp[crow, :], rb[crow, :])

            # ---- prologue: qt(0) --------------------------------------
            qts = [qpool.tile([128, E8, 512], BF16, tag="qt", name=f"qt{i}")
                   for i in range(2)]
            cts = [cpool.tile([128, E8, 512], BF16, tag="ct", name=f"ct{i}")
                   for i in range(2)]
            for e in range(E8):
                qproj_chunk(qts[0], 0, e)

            # ---- software-pipelined main loop -------------------------
            # groups: (tn, p); group i issues its scores/exp while group
            # i-1's PV accumulates between them; out-proj of tn-1 and
            # Q-proj of tn+1 chunks fill remaining PE slots.
            groups = [(tn, p) for tn in range(TN) for p in range(NP)]
            out_q = []    # pending outproj chunks: (ct, tn, m, n)
            qp_q = []     # pending qproj chunks: (qt, tn, e)
            prev = None   # (p_prev, at_tiles, cps, ct_prev)

            for tn, p in groups:
                qt = qts[tn % 2]
                ct = cts[tn % 2]
                if p == 0:
                    if tn + 1 < TN:
                        qp_q.extend((qts[(tn + 1) % 2], tn + 1, e)
                                    for e in range(E8))
                    if tn >= 1:
                        out_q.extend((cts[(tn - 1) % 2], tn - 1, m, n)
                                     for m in range(4) for n in range(NH))

                at_tiles = []
                cps = None
                if prev is not None:
                    cps = [cpsum.tile([128, 512], F32, tag="cp",
                                      name=f"cp_{tn}_{p}_{i}") for i in range(2)]

                # s8=0 scores first, then the Q-proj filler (no dependency
                # on this tn's ct) absorbs the cpsum recycle latency before
                # the first PV step needs the banks
                at_tiles.append(scores_exp(qt, p, 0))
                if qp_q:
                    qproj_chunk(*qp_q.pop(0))
                for s8 in range(1, S8):
                    if prev is not None:
                        pv_step(cps, prev[1], prev[0], s8 - 1)
                    at_tiles.append(scores_exp(qt, p, s8))
                if prev is not None:
                    pv_step(cps, prev[1], prev[0], S8 - 1)
                    normalize(prev[2], cps, prev[0])
                # out-proj chunks of tn-1 are only safe after normalize of
                # the pair that completed this group (they contract over
                # every pair), so they fill the post-normalize slot
                if out_q:
                    outproj_chunk(*out_q.pop(0))
                prev = (p, at_tiles, ct)

            # ---- epilogue: last group's PV + final out-proj -----------
            cps = [cpsum.tile([128, 512], F32, tag="cp", name=f"cp_last_{i}")
                   for i in range(2)]
            for s8 in range(S8):
                pv_step(cps, prev[1], prev[0], s8)
            normalize(prev[2], cps, prev[0])
            for m in range(4):
                for n in range(NH):
                    outproj_chunk(cts[(TN - 1) % 2], TN - 1, m, n)

    nc.compile()
    return nc


def _get_nc():
    if "nc" not in _BUILT:
        _BUILT["nc"] = _build_nc()
    return _BUILT["nc"]


def _bf16(x):
    return np.asarray(x, dtype=np.float32).astype(ml_dtypes.bfloat16)


def kernel(tokens, context, Wq, Wk, Wv, Wo, bo):
    tokens = np.asarray(tokens, dtype=np.float32).reshape(B * T, HID)
    context = np.asarray(context, dtype=np.float32)
    bo2 = np.asarray(bo, dtype=np.float32).reshape(1, HID)
    wq_b, wk_b, wv_b, wo_b = _bf16(Wq), _bf16(Wk), _bf16(Wv), _bf16(Wo)

    in_maps = []
    for c in range(N_CORES):
        b = (c * TC) // T
        tok_slice = tokens[c * TC:(c + 1) * TC, :]
        in_maps.append({
            "tokT": np.ascontiguousarray(tok_slice.T).astype(ml_dtypes.bfloat16),
            "ctxT": np.ascontiguousarray(context[b].T).astype(ml_dtypes.bfloat16),
            "wq": wq_b, "wk": wk_b, "wv": wv_b, "wo": wo_b, "bo": bo2,
        })

    nc = _get_nc()
    res = run_bass_kernel_spmd(nc, in_maps, list(range(N_CORES)))
    out = np.concatenate([res.results[c]["out"] for c in range(N_CORES)], axis=0)
    return out.reshape(B, T, HID)
